# revision 1
# baseline (speedup 1.0000x reference)
"""Trainium2 Bass kernel for MoGNN forward (global mean-pool + linear).

The model's conv outputs are discarded; the result depends only on x:
    pooled[g] = mean over nodes n with batch[n] == g of x[n]   # [1024, 512]
    out = pooled @ W.T + b                                     # [1024, 7]

batch ids are sorted, so nodes of each graph are contiguous. We shard by
GRAPHS: core k owns graphs [128k, 128k+128) and exactly the contiguous row
range of x belonging to them (padded to a tile multiple). No collectives.

Transport is mixed-precision to cut HBM traffic below the fp16 roofline:
each 708-byte row record is [graph label f16 x2 | fp16 x 192 | int8 x 320]
(int8 uses a global scale; x ~ N(0,1), clip at 4 sigma), one sequential DMA
stream per shard on the sync ring. The final two chunks travel as pure fp16
so the tail has no dequant dependency. Measured end-to-end rel err ~6e-3 vs
the 2e-2 gate.

Per 8-tile chunk, on device:
  - Activation engine dequantizes int8 cols [0,192) -> fp16 (scale folded
    into the copy); DVE dequantizes cols [192,320). Both write one xdq tile.
  - DVE builds the exact one-hot oh[n, g] = (label[n] == iota[g]) in 2x mode
    (all operands 2-byte with packed pair last-dims; labels are read straight
    from the record heads, iota is generated on-device by gpsimd) so the
    one-hot costs ~0.6us/chunk and no constants DMA gates the pipeline.
  - Per node tile, two PE matmuls accumulate into separate PSUM banks
    (interleaved accumulation groups sharing a bank corrupt each other on
    HW): acc16[128g, 0:192] += oh.T @ x_fp16, acc8[128g, 0:320] += oh.T @
    dequant(x_int8); the second reuses the loaded one-hot stationary
    (ldweights=False) to halve weight-load exposure.
Epilogue: per-bank PSUM -> SBUF scale by 1/count (mean pool), 4x PE
transpose to feat-major, 4 fp16 matmuls with pooled.T stationary and the W
chunk moving (N=7, fp32 PSUM), bias via a partition-replicated fp32 tile;
each core writes out[128, 7] and the host concatenates to [1024, 7].
"""

import numpy as np

NCORES = 8
G = 1024            # total graphs
GPC = G // NCORES   # graphs per core = 128
F = 512             # feature dim
F16C = 192          # columns shipped as fp16
ACT8C = 192         # int8 columns dequantized on the Activation engine
I8C = F - F16C      # columns shipped as int8
ROWB = 2 * F16C + I8C   # bytes per row record = 704
QSCALE = 4.0 / 127.0    # int8 quant scale for N(0,1) data, clip at 4 sigma
P = 128             # partition / node-tile size
CHUNK = 8           # node tiles per DMA chunk (768 KB transfers)
DQG = 8             # tiles per dequant group (Act-engine granularity)

_compiled_cache = {}


def _chunk_plan(ntiles):
    """Chunk boundaries: small leading chunks so the PE pipeline starts early,
    CHUNK-tile steady state, and a small taper at the end so the PE finishes
    right behind the final DMA bytes."""
    head = [min(2, CHUNK), min(6, CHUNK)]
    tail = [min(2, CHUNK)]
    main_end = max(ntiles - sum(tail), 0)
    chunks = []
    t0 = 0
    for ramp in head:
        if t0 < main_end:
            clen = min(ramp, main_end - t0)
            chunks.append((t0, clen))
            t0 += clen
    while t0 < main_end:
        clen = min(CHUNK, main_end - t0)
        chunks.append((t0, clen))
        t0 += clen
    for ramp in tail:
        if t0 < ntiles:
            clen = min(ramp, ntiles - t0)
            chunks.append((t0, clen))
            t0 += clen
    while t0 < ntiles:
        clen = min(CHUNK, ntiles - t0)
        chunks.append((t0, clen))
        t0 += clen
    assert sum(c for _, c in chunks) == ntiles
    # final two chunks travel as pure fp16: their matmuls then depend only on
    # the DMA, not on the Activation-engine dequant (which lags each chunk by
    # ~2us), so the PE finishes right behind the last bytes
    return [(c0, clen, ci >= len(chunks) - 2) for ci, (c0, clen) in enumerate(chunks)]


def _build(ntiles):
    """Build + compile the per-core Bass kernel for a shard of `ntiles` node tiles."""
    from concourse import bacc, tile, mybir

    f32 = mybir.dt.float32
    f16 = mybir.dt.float16
    i8 = mybir.dt.int8
    u8 = mybir.dt.uint8
    eq = mybir.AluOpType.is_equal
    mult = mybir.AluOpType.mult
    add = mybir.AluOpType.add

    chunks = _chunk_plan(ntiles)
    lb = 4 * ntiles  # label block bytes/partition: pair-duplicated f16 labels
    xs_bytes = P * lb + sum(
        clen * P * (2 * F if wide else ROWB) for _, clen, wide in chunks
    )

    nc = bacc.Bacc(
        "TRN2",
        target_bir_lowering=False,
        debug=False,
        num_devices=NCORES,
    )

    # x shard laid out chunk-contiguous and partition-major inside each chunk:
    # for chunk (c0, clen), the DRAM block holds block[p, t, b] (b a byte index
    # into the 768-byte row record), so the whole chunk is one contiguous
    # region and each partition reads one contiguous multi-KB run
    x_d = nc.dram_tensor("xs", [xs_bytes], u8, kind="ExternalInput")
    # constants packed into two tensors (one DMA each, on the scalar-engine
    # HWDGE ring so they don't block the x-chunk FIFO on the sync ring):
    # cp16 = [bl | iota | ident | wtr], cp32 = [b_replicated | icnt]
    cp16_d = nc.dram_tensor(
        "cp16", [P, P + 28 + 16], f16, kind="ExternalInput"
    )
    out_d = nc.dram_tensor("out", [7, GPC], f32, kind="ExternalOutput")

    with tile.TileContext(nc) as tc:
        with (
            tc.tile_pool(name="const", bufs=1) as constp,
            tc.tile_pool(name="xin", bufs=6) as xp,
            tc.tile_pool(name="xdq", bufs=8) as xdqp,
            tc.tile_pool(name="oh", bufs=10) as ohp,
            tc.tile_pool(name="acc16", bufs=1, space="PSUM") as accp16,
            tc.tile_pool(name="acc8", bufs=1, space="PSUM") as accp8,
            tc.tile_pool(name="tps", bufs=2, space="PSUM") as tpsp,
            tc.tile_pool(name="outp", bufs=1, space="PSUM") as outpp,
            tc.tile_pool(name="sb", bufs=2) as sbp,
        ):
            cp16_t = constp.tile([P, P + 28 + 16], f16)
            nc.scalar.dma_start(cp16_t[:], cp16_d.ap())
            cp32_t = cp16_t[:, P + 28 : P + 28 + 16].bitcast(f32)
            ident_t = cp16_t[:, 0:P]
            wtr_t = cp16_t[:, P : P + 28]
            # iota 0..127 generated on-device (exact in fp16): the one-hot
            # then has no dependency on any constants DMA
            iota_t = constp.tile([P, GPC], f16)
            nc.gpsimd.iota(
                iota_t[:], [[1, GPC]], base=0, channel_multiplier=0,
                allow_small_or_imprecise_dtypes=True,
            )
            bT_t = cp32_t[0:7, 0:1]    # b[j] on partition j, j < 7
            icnt_t = cp32_t[:, 7:8]

            # one full PSUM bank per accumulation group: interleaved groups
            # sharing a bank corrupt each other on HW (measured); separate
            # banks interleave cleanly
            acc16 = accp16.tile([GPC, F], f32)
            acc8 = accp8.tile([GPC, F], f32)
            x_flat = x_d.ap()

            # operand shapes chosen so every non-scalar AP has a packed
            # (stride-1, count-2) last dim: DVE then runs is_equal in 2x mode
            iota_pair = iota_t.rearrange("p (a g2 j) -> p a g2 j", a=1, j=2)
            t = 0
            byte_off = 0
            bl2_t = None
            for ci, (c0, clen, wide) in enumerate(chunks):
                rowb = 2 * F if wide else ROWB
                if ci == 0:
                    # chunk0 carries the whole shard's graph labels as a
                    # prefix, in one persistent tile and ONE transfer: every
                    # one-hot then depends only on this chunk, so the DVE
                    # builds them several chunks ahead of the PE
                    xt = constp.tile([P, lb + CHUNK * rowb], u8)
                    chunk_ap = x_flat[0 : P * (lb + clen * rowb)].rearrange(
                        "(p b) -> p b", p=P
                    )
                    byte_off = P * (lb + clen * rowb)
                    nc.sync.dma_start(xt[:, 0 : lb + clen * rowb], chunk_ap)
                    bl2_t = xt[:, 0:lb].bitcast(f16)        # [P, 2*ntiles]
                    recs0 = xt[:, lb : lb + clen * rowb].rearrange(
                        "p (t b) -> p t b", b=rowb
                    )
                    xt16 = recs0[:, :, 0 : 2 * F16C].bitcast(f16)
                    xt8a = recs0[:, :, 2 * F16C : 2 * F16C + ACT8C].bitcast(i8)
                    xt_for_dve = recs0
                else:
                    xt = xp.tile([P, CHUNK, rowb], u8, tag="xtw" if wide else "xt")
                    chunk_ap = x_flat[byte_off : byte_off + clen * P * rowb].rearrange(
                        "(p t b) -> p t b", p=P, b=rowb
                    )
                    byte_off += clen * P * rowb
                    nc.sync.dma_start(xt[:, :clen, :], chunk_ap)
                    xt_for_dve = xt
                    if wide:
                        xt16 = xt[:, :, 0 : 2 * F].bitcast(f16)
                    else:
                        # views into the packed record: fp16 / int8 blocks
                        xt16 = xt[:, :, 0 : 2 * F16C].bitcast(f16)
                        xt8a = xt[:, :, 2 * F16C : 2 * F16C + ACT8C].bitcast(i8)
                if not wide:
                    # dequantize the int8 block: the first ACT8C columns on the
                    # (otherwise idle) Activation engine, the rest on the DVE's
                    # slack; the quant scale folds into both copies
                    xdq = xdqp.tile([P, CHUNK, I8C], f16, tag="xdq")
                    nc.scalar.activation(
                        xdq[:, :clen, 0:ACT8C],
                        xt8a[:, :clen, :],
                        mybir.ActivationFunctionType.Copy,
                        scale=float(QSCALE),
                    )
                    if I8C > ACT8C:
                        xt8d = xt_for_dve[:, :, 2 * F16C + ACT8C : ROWB].bitcast(i8)
                        nc.vector.tensor_scalar(
                            xdq[:, :clen, ACT8C:I8C],
                            xt8d[:, :clen, :],
                            float(QSCALE),
                            None,
                            op0=mult,
                        )
                # one-hot for the whole chunk in one DVE op via broadcast APs:
                # oh[p, n, g] = (iota[g] == bl[p, c0+n])
                oh = ohp.tile([P, CHUNK, GPC], f16, tag="oh")
                nc.vector.tensor_tensor(
                    oh[:, :clen, :].rearrange("p n (g2 j) -> p n g2 j", j=2),
                    iota_pair.broadcast_to([P, clen, GPC // 2, 2]),
                    bl2_t[:, 2 * c0 : 2 * (c0 + clen)]
                    .rearrange("p (n a j) -> p n a j", a=1, j=2)
                    .broadcast_to([P, clen, GPC // 2, 2]),
                    op=eq,
                )
                # per tile: fp16-half matmul loads the one-hot stationary,
                # the int8-half matmul reuses it (ldweights=False) — halves
                # the PE's weight-load exposure for short N=256 matmuls
                for n in range(clen):
                    nc.tensor.matmul(
                        acc16[:, 0:F16C],
                        oh[:, n, :],
                        xt16[:, n, 0:F16C],
                        start=(t + n == 0),
                        stop=(t + n == ntiles - 1),
                        skip_group_check=True,
                    )
                    mmb = nc.tensor.matmul(
                        acc8[:, 0:I8C],
                        oh[:, n, :],
                        xt16[:, n, F16C:F] if wide else xdq[:, n, :],
                        start=(t + n == 0),
                        stop=(t + n == ntiles - 1),
                        skip_group_check=True,
                    )
                    mmb.ins.ldweights = False
                t += clen

            # pooled = acc * (1/count[g]) cast to fp16, sliced so the (fp16,
            # full-rate) transposes pipeline behind the scale copies; then the
            # classifier with pooled.T as stationary (moving is W [128, 7], N=7)
            pooled = sbp.tile([GPC, F], f16)
            ptall = sbp.tile([P, 4, P], f16)
            # mean-pool scale, one copy per accumulation bank (the fp16/int8
            # column split need not align to the 128-wide transpose blocks);
            # the fp16 bank goes through the Activation engine (idle by now)
            # so both scale copies run in parallel
            nc.scalar.activation(
                pooled[:, 0:F16C],
                acc16[:, 0:F16C],
                mybir.ActivationFunctionType.Copy,
                scale=icnt_t,
            )
            nc.vector.tensor_scalar(
                pooled[:, F16C:F], acc8[:, 0:I8C], icnt_t, None, op0=mult
            )
            for j in range(4):
                sl = slice(j * P, (j + 1) * P)
                tp = tpsp.tile([P, P], f16)
                nc.tensor.transpose(tp[:], pooled[:, sl], ident_t)
                nc.vector.tensor_copy(ptall[:, j, :], tp[:])

            # transposed classifier: W chunk stationary (M=7), pooled.T
            # moving -> out.T [7, 128]; the 3.5KB output then needs only 7
            # DMA descriptors instead of 128, cutting the end-of-kernel
            # descriptor-distribution latency before the final barrier
            out_ps = outpp.tile([7, GPC], f32)
            for j in range(4):
                nc.tensor.matmul(
                    out_ps[:],
                    wtr_t[:, j * 7 : (j + 1) * 7],
                    ptall[:, j, :],
                    start=(j == 0),
                    stop=(j == 3),
                )

            out_sb = sbp.tile([7, GPC], f32)
            nc.vector.tensor_scalar(out_sb[:], out_ps[:], bT_t, None, op0=add)
            nc.sync.dma_start(out_d.ap(), out_sb[:])

    nc.compile()
    return nc


def _get_compiled(ntiles):
    if ntiles not in _compiled_cache:
        _compiled_cache[ntiles] = _build(ntiles)
    return _compiled_cache[ntiles]


def _prep_in_maps(x32, batch, W, b, ntiles, bounds, inv_counts):
    cap = ntiles * P
    chunk_plan = _chunk_plan(ntiles)
    iota = np.tile(np.arange(GPC, dtype=np.float16)[None, :], (P, 1))
    # wtr[p, c*7+j] = W.T[c*128+p, j]
    wtr = np.ascontiguousarray(
        W.T.reshape(4, P, 7).transpose(1, 0, 2).reshape(P, 28)
    ).astype(np.float16)
    cp32_base = np.zeros((P, 8), dtype=np.float32)
    cp32_base[0:7, 0] = b.astype(np.float32)

    in_maps = []
    for k in range(NCORES):
        lo, hi = int(bounds[k]), int(bounds[k + 1])
        n = hi - lo
        shard = x32[lo:hi]
        blv = np.full((cap,), -1.0, dtype=np.float16)
        blv[:n] = (batch[lo:hi] - GPC * k).astype(np.float16)
        # pair-duplicated label block [P, 2*ntiles], rides ahead of chunk0
        blt = blv.reshape(ntiles, P).T
        labels = np.empty((P, 2 * ntiles), dtype=np.float16)
        labels[:, 0::2] = blt
        labels[:, 1::2] = blt
        xf = np.zeros((cap, F), dtype=np.float16)
        xf[:n] = shard.astype(np.float16)
        packed = np.zeros((cap, ROWB), dtype=np.uint8)
        packed[:, 0 : 2 * F16C] = xf[:, 0:F16C].view(np.uint8)
        q = np.zeros((cap, I8C), dtype=np.int8)
        q[:n] = np.clip(
            np.round(shard[:, F16C:F] / QSCALE), -127, 127
        ).astype(np.int8)
        packed[:, 2 * F16C : ROWB] = q.view(np.uint8)
        packed = packed.reshape(ntiles, P, ROWB)
        wide_rows = xf.view(np.uint8).reshape(ntiles, P, 2 * F)
        # chunk-contiguous, partition-major within each chunk; chunk0 gets the
        # label block prefixed per partition; the final chunks are pure fp16
        parts = []
        for ci, (c0, clen, wide) in enumerate(chunk_plan):
            blk = np.ascontiguousarray(
                (wide_rows if wide else packed)[c0 : c0 + clen].transpose(1, 0, 2)
            ).reshape(P, -1)
            if ci == 0:
                blk = np.concatenate([labels.view(np.uint8), blk], axis=1)
            parts.append(blk.reshape(-1))
        xs = np.concatenate(parts)
        cp16 = np.empty((P, P + 28 + 16), dtype=np.float16)
        cp16[:, 0:P] = np.eye(P, dtype=np.float16)
        cp16[:, P : P + 28] = wtr
        cp32 = cp32_base.copy()
        cp32[:, 7] = inv_counts[GPC * k : GPC * (k + 1)]
        cp16[:, P + 28 :] = cp32.view(np.float16)
        in_maps.append({"xs": xs, "cp16": cp16})
    return in_maps


_last_result = None  # test harness can read exec_time_ns / trace from here


def kernel(x, edge_index, edge_attr, batch_size, W, b):
    from concourse import bass_utils

    global _last_result

    x32 = np.asarray(x, dtype=np.float32)
    batch = np.asarray(batch_size).astype(np.int64)
    W = np.asarray(W, dtype=np.float32)
    b = np.asarray(b, dtype=np.float32)

    if batch.size > 1 and np.any(np.diff(batch) < 0):
        # contiguous-shard logic needs sorted ids; reordering nodes does not
        # change per-graph sums
        order = np.argsort(batch, kind="stable")
        batch = batch[order]
        x32 = x32[order]

    counts = np.bincount(batch, minlength=G)
    inv_counts = (1.0 / np.maximum(counts, 1)).astype(np.float32)
    bounds = np.searchsorted(batch, np.arange(0, G + 1, GPC))
    max_rows = int(np.diff(bounds).max())
    ntiles = max(-(-max_rows // P), 1)

    nc = _get_compiled(ntiles)
    in_maps = _prep_in_maps(x32, batch, W, b, ntiles, bounds, inv_counts)

    res = bass_utils.run_bass_kernel_spmd(
        nc, in_maps, core_ids=list(range(NCORES))
    )
    _last_result = res

    # each core returns out.T [7, 128] for its graphs; assemble [1024, 7]
    out = np.concatenate(
        [np.asarray(res.results[k]["out"]) for k in range(NCORES)], axis=1
    ).T
    return np.ascontiguousarray(out.astype(np.float32))



# revision 3
# speedup vs baseline: 1.0078x; 1.0078x over previous
"""Trainium2 Bass kernel for MoGNN forward (global mean-pool + linear).

The model's conv outputs are discarded; the result depends only on x:
    pooled[g] = mean over nodes n with batch[n] == g of x[n]   # [1024, 512]
    out = pooled @ W.T + b                                     # [1024, 7]

batch ids are sorted, so nodes of each graph are contiguous. We shard by
GRAPHS: core k owns graphs [128k, 128k+128) and exactly the contiguous row
range of x belonging to them. No collectives.

Transport is pure int8 (global scale, 4-sigma clip; measured end-to-end rel
err ~9e-3 vs the 2e-2 gate), 512B per node row -- the DMA-byte floor without
sub-byte unpack work. Rows ride as SAME-GRAPH PAIRS (each graph's row range
is padded to an even count host-side): one 1024-byte record = [even row int8
x512 | odd row int8 x512]. A pair-tile is 128 records.

Per pair-tile the reduction is split by feature columns so every engine
stays under the ~360ns/tile DMA-bus floor:
  - cols [0,256):  DVE scalar_tensor_tensor (even*1.0)+odd -> fp16 pair-sum
    (int8 adds are exact in fp16; the quant scale folds into the mean-pool
    epilogue constant).
  - cols [256,320): same fused pair-add on the (otherwise idle) Pool engine.
  - cols [320,512): Activation engine dequantizes even and odd halves to
    fp16; the PE consumes both rows (the pair shares one one-hot).
  - DVE builds the exact one-hot oh[n, g] = (pairlabel[n] == iota[g]) in 2x
    mode (all operands 2-byte with packed pair last-dims); iota is generated
    on-device by gpsimd so no constants DMA gates the pipeline.
  - PE per tile: mm1 acc_pair[128g, 0:320] += oh.T @ pairsum (ldweights),
    mm2/mm3 acc_eo[128g, 0:192] += oh.T @ xeo[even/odd] reusing the loaded
    one-hot (ldweights=False). Separate PSUM banks per accumulation group
    (interleaved groups sharing a bank corrupt each other on HW).
A short burst of dummy PE transposes at kernel start ramps the tensor
engine out of its low p-state during the first-chunk DMA latency window.

Epilogue: per-bank PSUM -> SBUF scale by QSCALE/count (mean pool), 4x PE
transpose to feat-major, 4 fp16 matmuls with the W chunk stationary (N=7,
fp32 PSUM), bias via a partition-replicated fp32 tile; each core writes
out.T [7, 128] (7 DMA descriptors) and the host concatenates to [1024, 7].
"""

import numpy as np

NCORES = 8
G = 1024            # total graphs
GPC = G // NCORES   # graphs per core = 128
F = 512             # feature dim
REC = 2 * F         # bytes per pair record (two int8 rows)
QSCALE = 4.0 / 127.0    # int8 quant scale for N(0,1) data, clip at 4 sigma
P = 128             # partition / pair-tile size
CHUNK = 8           # pair-tiles per DMA chunk (1 MB transfers)
D_DVE = 256         # feature cols pair-added on the DVE
P_POOL = 64         # feature cols pair-added on the Pool engine
C_ACT = F - D_DVE - P_POOL  # cols dequantized (even+odd) on Activation
NWARM = 16          # dummy PE ops to ramp the p-state during DMA latency

_compiled_cache = {}


def _chunk_plan(ntiles):
    """Small leading chunks so the pipeline starts early, then CHUNK-tile
    steady state; the natural remainder gives a small tail chunk."""
    chunks = []
    t0 = 0
    for ramp in (2, 6):
        if t0 < ntiles:
            clen = min(ramp, ntiles - t0)
            chunks.append((t0, clen))
            t0 += clen
    while t0 < ntiles:
        clen = min(CHUNK, ntiles - t0)
        chunks.append((t0, clen))
        t0 += clen
    assert sum(c for _, c in chunks) == ntiles
    return chunks


def _build(ntiles):
    """Build + compile the per-core Bass kernel for `ntiles` pair-tiles."""
    from concourse import bacc, tile, mybir

    f32 = mybir.dt.float32
    f16 = mybir.dt.float16
    i8 = mybir.dt.int8
    u8 = mybir.dt.uint8
    eq = mybir.AluOpType.is_equal
    mult = mybir.AluOpType.mult
    add = mybir.AluOpType.add

    chunks = _chunk_plan(ntiles)
    lb = 4 * ntiles  # label block bytes/partition: pair-duplicated f16 labels
    xs_bytes = P * (lb + ntiles * REC)

    nc = bacc.Bacc(
        "TRN2",
        target_bir_lowering=False,
        debug=False,
        num_devices=NCORES,
    )

    # x shard laid out chunk-contiguous and partition-major inside each chunk:
    # for chunk (c0, clen) the DRAM block holds block[p, t, b] (b a byte index
    # into the 1024-byte pair record), so the whole chunk is one contiguous
    # region and each partition reads one contiguous multi-KB run
    x_d = nc.dram_tensor("xs", [xs_bytes], u8, kind="ExternalInput")
    # constants packed into one tensor (single DMA on the scalar-engine ring
    # so it doesn't block the x-chunk FIFO on the sync ring):
    # cp16 = [ident | wtr | cp32(b_replicated, qscale/count)]
    cp16_d = nc.dram_tensor(
        "cp16", [P, P + 28 + 16], f16, kind="ExternalInput"
    )
    out_d = nc.dram_tensor("out", [7, GPC], f32, kind="ExternalOutput")

    with tile.TileContext(nc) as tc:
        with (
            tc.tile_pool(name="const", bufs=1) as constp,
            tc.tile_pool(name="xin", bufs=4) as xp,
            tc.tile_pool(name="ps", bufs=4) as psp,
            tc.tile_pool(name="xeo", bufs=4) as xeop,
            tc.tile_pool(name="oh", bufs=6) as ohp,
            tc.tile_pool(name="accp", bufs=1, space="PSUM") as accpp,
            tc.tile_pool(name="acce", bufs=1, space="PSUM") as accep,
            tc.tile_pool(name="warm", bufs=1, space="PSUM") as warmp,
            tc.tile_pool(name="tps", bufs=2, space="PSUM") as tpsp,
            tc.tile_pool(name="outp", bufs=1, space="PSUM") as outpp,
            tc.tile_pool(name="sb", bufs=2) as sbp,
        ):
            cp16_t = constp.tile([P, P + 28 + 16], f16)
            nc.scalar.dma_start(cp16_t[:], cp16_d.ap())
            cp32_t = cp16_t[:, P + 28 : P + 28 + 16].bitcast(f32)
            ident_t = cp16_t[:, 0:P]
            wtr_t = cp16_t[:, P : P + 28]
            bT_t = cp32_t[0:7, 0:1]    # b[j] on partition j, j < 7
            icnt_t = cp32_t[:, 7:8]    # QSCALE / max(count, 1) per graph

            # iota 0..127 generated on-device (exact in fp16): the one-hot
            # then has no dependency on any constants DMA
            iota_t = constp.tile([P, GPC], f16)
            nc.gpsimd.iota(
                iota_t[:], [[1, GPC]], base=0, channel_multiplier=0,
                allow_small_or_imprecise_dtypes=True,
            )

            # PE p-state warmup: dummy matmuls on a zeroed tile keep the
            # tensor engine busy through the first-chunk DMA latency so real
            # matmuls run at full clock. No data dependencies.
            wz = constp.tile([P, P], f16)
            nc.vector.memset(wz[:], 0)
            warm_t = warmp.tile([P, P], f32)
            for _ in range(NWARM):
                nc.tensor.matmul(warm_t[:], wz[:], wz[:], start=True, stop=True)

            # one full PSUM bank per accumulation group: interleaved groups
            # sharing a bank corrupt each other on HW; separate banks
            # interleave cleanly
            acc_pair = accpp.tile([GPC, F], f32)
            acc_eo = accep.tile([GPC, F], f32)
            x_flat = x_d.ap()

            # operand shapes chosen so every non-scalar AP has a packed
            # (stride-1, count-2) last dim: DVE then runs is_equal in 2x mode
            iota_pair = iota_t.rearrange("p (a g2 j) -> p a g2 j", a=1, j=2)
            t = 0
            byte_off = 0
            bl2_t = None
            for ci, (c0, clen) in enumerate(chunks):
                if ci == 0:
                    # chunk0 carries the whole shard's pair labels as a
                    # prefix, in one persistent tile and ONE transfer: every
                    # one-hot then depends only on this chunk, so the DVE
                    # builds them several chunks ahead of the PE
                    xt = constp.tile([P, lb + CHUNK * REC], u8)
                    chunk_ap = x_flat[0 : P * (lb + clen * REC)].rearrange(
                        "(p b) -> p b", p=P
                    )
                    byte_off = P * (lb + clen * REC)
                    nc.sync.dma_start(xt[:, 0 : lb + clen * REC], chunk_ap)
                    bl2_t = xt[:, 0:lb].bitcast(f16)        # [P, 2*ntiles]
                    recs = xt[:, lb : lb + clen * REC].rearrange(
                        "p (t b) -> p t b", b=REC
                    )
                else:
                    xt = xp.tile([P, CHUNK, REC], u8, tag="xt")
                    chunk_ap = x_flat[byte_off : byte_off + clen * P * REC].rearrange(
                        "(p t b) -> p t b", p=P, b=REC
                    )
                    byte_off += clen * P * REC
                    nc.sync.dma_start(xt[:, :clen, :], chunk_ap)
                    recs = xt
                even = recs[:, :, 0:F].bitcast(i8)
                odd = recs[:, :, F:REC].bitcast(i8)

                # fused pair-add + dequant: (even * 1.0) + odd -> fp16, exact
                # for int8 sums; quant scale folds into the epilogue constant
                ps = psp.tile([P, CHUNK, D_DVE + P_POOL], f16, tag="ps")
                nc.vector.scalar_tensor_tensor(
                    ps[:, :clen, 0:D_DVE],
                    even[:, :clen, 0:D_DVE],
                    1.0,
                    odd[:, :clen, 0:D_DVE],
                    op0=mult,
                    op1=add,
                )
                nc.gpsimd.tensor_tensor(
                    ps[:, :clen, D_DVE : D_DVE + P_POOL],
                    even[:, :clen, D_DVE : D_DVE + P_POOL],
                    odd[:, :clen, D_DVE : D_DVE + P_POOL],
                    op=add,
                )
                # Activation engine dequantizes the remaining columns of both
                # pair halves; the PE adds them via two matmuls on one one-hot
                xeo = xeop.tile([P, CHUNK, 2, C_ACT], f16, tag="xeo")
                nc.scalar.activation(
                    xeo[:, :clen, 0, :],
                    even[:, :clen, D_DVE + P_POOL : F],
                    mybir.ActivationFunctionType.Copy,
                    scale=1.0,
                )
                nc.scalar.activation(
                    xeo[:, :clen, 1, :],
                    odd[:, :clen, D_DVE + P_POOL : F],
                    mybir.ActivationFunctionType.Copy,
                    scale=1.0,
                )
                # one-hot for the whole chunk in one DVE op via broadcast APs:
                # oh[p, n, g] = (iota[g] == pairlabel[p, c0+n])
                oh = ohp.tile([P, CHUNK, GPC], f16, tag="oh")
                nc.vector.tensor_tensor(
                    oh[:, :clen, :].rearrange("p n (g2 j) -> p n g2 j", j=2),
                    iota_pair.broadcast_to([P, clen, GPC // 2, 2]),
                    bl2_t[:, 2 * c0 : 2 * (c0 + clen)]
                    .rearrange("p (n a j) -> p n a j", a=1, j=2)
                    .broadcast_to([P, clen, GPC // 2, 2]),
                    op=eq,
                )
                # per tile: the pair-sum matmul loads the one-hot stationary,
                # the even/odd matmuls reuse it (ldweights=False)
                for n in range(clen):
                    nc.tensor.matmul(
                        acc_pair[:, 0 : D_DVE + P_POOL],
                        oh[:, n, :],
                        ps[:, n, :],
                        start=(t + n == 0),
                        stop=(t + n == ntiles - 1),
                        skip_group_check=True,
                    )
                    mm2 = nc.tensor.matmul(
                        acc_eo[:, 0:C_ACT],
                        oh[:, n, :],
                        xeo[:, n, 0, :],
                        start=(t + n == 0),
                        stop=False,
                        skip_group_check=True,
                    )
                    mm2.ins.ldweights = False
                    mm3 = nc.tensor.matmul(
                        acc_eo[:, 0:C_ACT],
                        oh[:, n, :],
                        xeo[:, n, 1, :],
                        start=False,
                        stop=(t + n == ntiles - 1),
                        skip_group_check=True,
                    )
                    mm3.ins.ldweights = False
                t += clen

            # pooled = acc * (QSCALE/count[g]) cast to fp16; the two banks
            # scale on different engines so the copies run in parallel
            pooled = sbp.tile([GPC, F], f16)
            ptall = sbp.tile([P, 4, P], f16)
            nc.vector.tensor_scalar(
                pooled[:, 0 : D_DVE + P_POOL],
                acc_pair[:, 0 : D_DVE + P_POOL],
                icnt_t,
                None,
                op0=mult,
            )
            nc.scalar.activation(
                pooled[:, D_DVE + P_POOL : F],
                acc_eo[:, 0:C_ACT],
                mybir.ActivationFunctionType.Copy,
                scale=icnt_t,
            )
            for j in range(4):
                sl = slice(j * P, (j + 1) * P)
                tp = tpsp.tile([P, P], f16)
                nc.tensor.transpose(tp[:], pooled[:, sl], ident_t)
                nc.vector.tensor_copy(ptall[:, j, :], tp[:])

            # transposed classifier: W chunk stationary (M=7), pooled.T
            # moving -> out.T [7, 128]; the 3.5KB output then needs only 7
            # DMA descriptors instead of 128
            out_ps = outpp.tile([7, GPC], f32)
            for j in range(4):
                nc.tensor.matmul(
                    out_ps[:],
                    wtr_t[:, j * 7 : (j + 1) * 7],
                    ptall[:, j, :],
                    start=(j == 0),
                    stop=(j == 3),
                )

            out_sb = sbp.tile([7, GPC], f32)
            nc.vector.tensor_scalar(out_sb[:], out_ps[:], bT_t, None, op0=add)
            nc.sync.dma_start(out_d.ap(), out_sb[:])

    nc.compile()
    return nc


def _get_compiled(ntiles):
    if ntiles not in _compiled_cache:
        _compiled_cache[ntiles] = _build(ntiles)
    return _compiled_cache[ntiles]


def _prep_in_maps(q, batch, W, b, ntiles, bounds, scale_g):
    cap_pairs = ntiles * P
    chunk_plan = _chunk_plan(ntiles)
    # wtr[p, c*7+j] = W.T[c*128+p, j]
    wtr = np.ascontiguousarray(
        W.T.reshape(4, P, 7).transpose(1, 0, 2).reshape(P, 28)
    ).astype(np.float16)
    cp32_base = np.zeros((P, 8), dtype=np.float32)
    cp32_base[0:7, 0] = b.astype(np.float32)

    in_maps = []
    for k in range(NCORES):
        lo, hi = int(bounds[k]), int(bounds[k + 1])
        n = hi - lo
        lbatch = (batch[lo:hi] - GPC * k).astype(np.int64)
        c = np.bincount(lbatch, minlength=GPC)
        c2 = c + (c & 1)          # pad each graph to an even row count
        off2 = np.zeros(GPC + 1, dtype=np.int64)
        np.cumsum(c2, out=off2[1:])
        starts = np.zeros(GPC + 1, dtype=np.int64)
        np.cumsum(c, out=starts[1:])
        R2 = int(off2[-1])
        assert R2 <= 2 * cap_pairs
        qrows = np.zeros((2 * cap_pairs, F), dtype=np.int8)
        dst = (np.arange(n) - starts[lbatch]) + off2[lbatch]
        qrows[dst] = q[lo:hi]
        # pair labels (graph of both rows of each pair; -1 pads)
        plab_full = np.full(cap_pairs, -1.0, dtype=np.float16)
        plab_full[: R2 // 2] = np.repeat(
            np.arange(GPC, dtype=np.float16), c2
        )[0::2]
        blt = plab_full.reshape(ntiles, P).T          # [P, ntiles]
        labels = np.empty((P, 2 * ntiles), dtype=np.float16)
        labels[:, 0::2] = blt
        labels[:, 1::2] = blt
        # pair records [cap_pairs, 1024] = [even | odd] int8 rows
        recs = qrows.view(np.uint8).reshape(ntiles, P, REC)
        parts = []
        for ci, (c0, clen) in enumerate(chunk_plan):
            blk = np.ascontiguousarray(
                recs[c0 : c0 + clen].transpose(1, 0, 2)
            ).reshape(P, -1)
            if ci == 0:
                blk = np.concatenate([labels.view(np.uint8), blk], axis=1)
            parts.append(blk.reshape(-1))
        xs = np.concatenate(parts)
        cp16 = np.empty((P, P + 28 + 16), dtype=np.float16)
        cp16[:, 0:P] = np.eye(P, dtype=np.float16)
        cp16[:, P : P + 28] = wtr
        cp32 = cp32_base.copy()
        cp32[:, 7] = scale_g[GPC * k : GPC * (k + 1)]
        cp16[:, P + 28 :] = cp32.view(np.float16)
        in_maps.append({"xs": xs, "cp16": cp16})
    return in_maps


_last_result = None  # test harness can read exec_time_ns / trace from here


def kernel(x, edge_index, edge_attr, batch_size, W, b):
    from concourse import bass_utils

    global _last_result

    x32 = np.asarray(x, dtype=np.float32)
    batch = np.asarray(batch_size).astype(np.int64)
    W = np.asarray(W, dtype=np.float32)
    b = np.asarray(b, dtype=np.float32)

    if batch.size > 1 and np.any(np.diff(batch) < 0):
        # contiguous-shard logic needs sorted ids; reordering nodes does not
        # change per-graph sums
        order = np.argsort(batch, kind="stable")
        batch = batch[order]
        x32 = x32[order]

    q = np.clip(np.round(x32 * (1.0 / QSCALE)), -127, 127).astype(np.int8)
    counts = np.bincount(batch, minlength=G)
    scale_g = (QSCALE / np.maximum(counts, 1)).astype(np.float32)
    bounds = np.searchsorted(batch, np.arange(0, G + 1, GPC))
    # per-core padded pair count: rows + one pad row per odd graph
    max_pairs = 0
    for k in range(NCORES):
        lb = batch[bounds[k] : bounds[k + 1]] - GPC * k
        c = np.bincount(lb, minlength=GPC)
        max_pairs = max(max_pairs, int((c + (c & 1)).sum()) // 2)
    ntiles = max(-(-max_pairs // P), 1)

    nc = _get_compiled(ntiles)
    in_maps = _prep_in_maps(q, batch, W, b, ntiles, bounds, scale_g)

    res = bass_utils.run_bass_kernel_spmd(
        nc, in_maps, core_ids=list(range(NCORES))
    )
    _last_result = res

    # each core returns out.T [7, 128] for its graphs; assemble [1024, 7]
    out = np.concatenate(
        [np.asarray(res.results[k]["out"]) for k in range(NCORES)], axis=1
    ).T
    return np.ascontiguousarray(out.astype(np.float32))
